# revision 3
# baseline (speedup 1.0000x reference)
"""VQ codebook lookup kernel for 8 Trainium2 NeuronCores.

Problem: for x [32,512,256] f32, codebook [4096,256] f32, random_indices [3686] i32
(sorted subset of codes), compute per token the nearest (L2) subset code:
    closest_tokens [32,512,256] f32, rounding_loss (scalar f32), closest_indices [32,512] i32

Strategy (data-parallel over the 16384 tokens, 2048/core):
  argmin_k ||x - c_k||^2 == argmax_k (x . c_k - 0.5||c_k||^2).  We compute scores
  over ALL 4096 codes with a -60000 penalty baked into the per-code bias for
  dropped codes, so the argmax index IS the global codebook index (no remap).
  - GEMM x @ c^T in bf16 hi/lo split (3 passes -> ~fp32 accuracy, bf16 speed).
  - Per-code bias (-0.5*||c||^2 + penalty) added inside a custom single-pass
    DVE argmax op (running-max record scan), which also returns the index.
  - Tokens gathered from the codebook via indirect DMA; loss via a fused
    (a-b)^2-sum custom DVE op; final cross-partition sum via a ones-matmul.
"""

import sys

if "/opt/trn_rl_repo" not in sys.path:
    sys.path.insert(0, "/opt/trn_rl_repo")

import numpy as np

import concourse.bass as bass
import concourse.tile as tile
import concourse.mybir as mybir
from concourse import bacc
from concourse.bass_utils import run_bass_kernel_spmd

import concourse.dve_ops as dve_ops
from concourse.dve_ops import has_src1
from concourse.dve_spec import (
    Spec, Src0, Src1, C0, MaxNeg, select, eq, sq, lower, AluOp, scan, Idx,
)
from concourse.dve_uop import DveOpSpec

# ---------------------------------------------------------------------------
# problem constants (hardcoded per harness contract)
B, S, D = 32, 512, 256
NTOK = 4096           # codebook size
NCORES = 8
N = B * S             # 16384 tokens
NLOC = N // NCORES    # 2048 tokens per core
RT = NLOC // 128      # 16 row-tiles per core
NCHUNK = NTOK // 512  # 8 code chunks of 512
KT = D // 128         # 2 contraction tiles
PENALTY = -60000.0

f32 = mybir.dt.float32
bf16 = mybir.dt.bfloat16
i32 = mybir.dt.int32


def _register_op(name, spec):
    """Register a custom DVE op at runtime (idempotent) and pin its sha."""
    for op in dve_ops.OPS:
        if op.name == name:
            return op
    op = dve_ops.DveOp(name, spec, subdim=False, uops_sha={})
    dve_ops.OPS.append(op)
    dve_ops._SUB_OPCODE_FOR_NAME[name] = (
        dve_ops._CUSTOM_DVE_ROW_BASE + len(dve_ops.OPS) - 1
    )
    for ver in ("v3",):
        uops = lower(op.spec, ver=ver)
        s = DveOpSpec(name=name, opcode=dve_ops.get_dve_sub_opcode(name),
                      uops=uops, rd1_en=has_src1(op.spec))
        op.uops_sha[ver] = s.sha(ver)
    return op


def _make_ops():
    # single-pass biased argmax: s = in0 + in1; accum_out = argmax_k s  (last
    # occurrence on exact fp32 ties; ties are measure-zero for this data)
    _s = Src0 + Src1
    _r = scan(AluOp.MAX, _s)

    def _ref_argmax(in0, in1, s0, s1, imm2):
        s = (in0.astype(np.float32) + in1.astype(np.float32)).astype(np.float32)
        r = np.maximum.accumulate(s, axis=1)
        idx = np.arange(s.shape[1], dtype=np.float32)[None, :]
        out = np.where(s == r, idx, -np.finfo(np.float32).max)
        return out.astype(np.float32), out.max(axis=1).astype(np.float32)

    argmax_op = _register_op(
        "VQ_ARGMAX_B",
        Spec(body=select(eq(_s, _r), Idx, MaxNeg), accum=AluOp.MAX,
             reference=_ref_argmax),
    )

    # fused squared-error reduce: accum_out = s0 + sum_k (in0-in1)^2
    def _ref_sse(in0, in1, s0, s1, imm2):
        d = in0.astype(np.float32) - in1.astype(np.float32)
        out = (d * d).astype(np.float32)
        acc = (np.asarray(s0, dtype=np.float32).reshape(-1)
               + out.sum(axis=1)).astype(np.float32)
        return out, acc

    sse_op = _register_op(
        "VQ_SSE_ACC",
        Spec(body=sq(Src0 - Src1), accum=AluOp.ADD, accum_init=C0,
             reference=_ref_sse),
    )
    return argmax_op, sse_op


def build_program():
    argmax_op, sse_op = _make_ops()
    nc = bacc.Bacc("TRN2", target_bir_lowering=False, debug=False,
                   num_devices=NCORES)

    xT_d = nc.dram_tensor("xT", [D, NLOC], f32, kind="ExternalInput")
    xn_d = nc.dram_tensor("xn", [NLOC, D], f32, kind="ExternalInput")
    cT_d = nc.dram_tensor("cT", [D, NTOK], f32, kind="ExternalInput")
    cb_d = nc.dram_tensor("cb", [NTOK, D], f32, kind="ExternalInput")
    pen_d = nc.dram_tensor("pen", [1, NTOK], f32, kind="ExternalInput")

    ctok_d = nc.dram_tensor("ctok", [NLOC, D], f32, kind="ExternalOutput")
    cidx_d = nc.dram_tensor("cidx", [NLOC], i32, kind="ExternalOutput")
    lpart_d = nc.dram_tensor("lpart", [1, 1], f32, kind="ExternalOutput")

    with tile.TileContext(nc) as tc:
        with tc.tile_pool(name="persist", bufs=1) as pp:
            # ---- persistent tiles ----------------------------------------
            xh = [pp.tile([128, NLOC], bf16, name=f"xh{k}") for k in range(KT)]
            xl = [pp.tile([128, NLOC], bf16, name=f"xl{k}") for k in range(KT)]
            ch = [pp.tile([128, NTOK], bf16, name=f"ch{k}") for k in range(KT)]
            cl = [pp.tile([128, NTOK], bf16, name=f"cl{k}") for k in range(KT)]
            bias_rep = pp.tile([128, NTOK], f32, name="bias_rep")
            idxf = pp.tile([128, RT], f32, name="idxf")
            idxi = pp.tile([128, RT], i32, name="idxi")
            ctok_big = pp.tile([128, RT * D], f32, name="ctok_big")
            xn_big = pp.tile([128, RT * D], f32, name="xn_big")
            ones_sb = pp.tile([128, 1], f32, name="ones_sb")
            sse_acc = pp.tile([128, 1], f32, name="sse_acc")
            lout = pp.tile([1, 1], f32, name="lout")

            with tc.tile_pool(name="staging", bufs=1) as sp, \
                 tc.tile_pool(name="psum_pre", bufs=1, space="PSUM") as ppre:
                xTf = sp.tile([128, NLOC], f32, name="xTf")
                cTf = sp.tile([128, NTOK], f32, name="cTf")
                sqc = sp.tile([128, NTOK], f32, name="sqc")
                pen_sb = sp.tile([1, NTOK], f32, name="pen_sb")
                c2_sb = sp.tile([1, NTOK], f32, name="c2_sb")
                onesw = sp.tile([128, 1], f32, name="onesw")

                nc.sync.dma_start(pen_sb[:], pen_d.ap())
                nc.vector.memset(onesw[:], 1.0)
                nc.vector.memset(ones_sb[:], 1.0)
                c2_ps = [ppre.tile([128, 512], f32, name=f"c2ps{j}")
                         for j in range(NCHUNK)]
                for k in range(KT):
                    nc.sync.dma_start(xTf[:], xT_d.ap()[k * 128:(k + 1) * 128, :])
                    nc.scalar.copy(xh[k][:], xTf[:])
                    nc.vector.tensor_sub(xl[k][:], xTf[:], xh[k][:])
                    nc.sync.dma_start(cTf[:], cT_d.ap()[k * 128:(k + 1) * 128, :])
                    nc.scalar.copy(ch[k][:], cTf[:])
                    nc.vector.tensor_sub(cl[k][:], cTf[:], ch[k][:])
                    nc.scalar.activation(sqc[:], cTf[:],
                                         mybir.ActivationFunctionType.Square)
                    # c2 += ones^T @ sqc  -> [1, NTOK]
                    for j in range(NCHUNK):
                        nc.tensor.matmul(
                            c2_ps[j][:1, :], onesw[:],
                            sqc[:, j * 512:(j + 1) * 512],
                            start=(k == 0), stop=(k == KT - 1))
                for j in range(NCHUNK):
                    nc.scalar.copy(c2_sb[:, j * 512:(j + 1) * 512], c2_ps[j][:1, :])
                # bias = -0.5*c2 + pen   (on one partition), then broadcast
                nc.vector.tensor_scalar_mul(c2_sb[:], c2_sb[:], -0.5)
                nc.vector.tensor_add(c2_sb[:], c2_sb[:], pen_sb[:])
                nc.gpsimd.partition_broadcast(bias_rep[:], c2_sb[:])

            # ---- main loop ------------------------------------------------
            with tc.tile_pool(name="scr", bufs=1) as scrp, \
                 tc.tile_pool(name="scores", bufs=2) as scp, \
                 tc.tile_pool(name="psum", bufs=8, space="PSUM") as psp:
                scratch = scrp.tile([128, NTOK], f32, name="scratch")
                for r in range(RT):
                    scores = scp.tile([128, NTOK], f32, name=f"scores{r}",
                                      tag="scores")
                    xs = slice(r * 128, (r + 1) * 128)
                    for j in range(NCHUNK):
                        ps = psp.tile([128, 512], f32, name=f"ps{r}_{j}", tag="ps")
                        cs = slice(j * 512, (j + 1) * 512)
                        mms = [(xh, ch), (xh, cl), (xl, ch)]
                        n_mm = len(mms) * KT
                        i = 0
                        for lhs_set, rhs_set in mms:
                            for k in range(KT):
                                nc.tensor.matmul(
                                    ps[:], lhs_set[k][:, xs], rhs_set[k][:, cs],
                                    start=(i == 0), stop=(i == n_mm - 1))
                                i += 1
                        nc.scalar.copy(scores[:, cs], ps[:])
                    # biased argmax over the 4096 codes
                    nc.vector._custom_dve(
                        argmax_op, out=scratch[:], in0=scores[:],
                        in1=bias_rep[:], accum_out=idxf[:, r:r + 1])
                    # index -> int32 (gpsimd to keep DVE free)
                    nc.gpsimd.tensor_copy(idxi[:, r:r + 1], idxf[:, r:r + 1])
                    # gather closest tokens + stream x for the loss
                    ds = slice(r * D, (r + 1) * D)
                    nc.gpsimd.indirect_dma_start(
                        out=ctok_big[:, ds], out_offset=None,
                        in_=cb_d.ap(),
                        in_offset=bass.IndirectOffsetOnAxis(
                            ap=idxi[:, r:r + 1], axis=0))
                    nc.sync.dma_start(xn_big[:, ds], xn_d.ap()[xs, :])
                    nc.sync.dma_start(ctok_d.ap()[xs, :], ctok_big[:, ds])

                # ---- epilogue: indices out + loss -------------------------
                nc.sync.dma_start(
                    cidx_d.ap().rearrange("(t p) -> p t", p=128), idxi[:])
                nc.vector._custom_dve(
                    sse_op, out=scratch[:, :RT * D], in0=ctok_big[:],
                    in1=xn_big[:], s0=0.0, accum_out=sse_acc[:])
                lps = psp.tile([1, 1], f32, name="lps", tag="ps")
                nc.tensor.matmul(lps[:], sse_acc[:], ones_sb[:],
                                 start=True, stop=True)
                nc.scalar.copy(lout[:], lps[:1, :])
                nc.sync.dma_start(lpart_d.ap(), lout[:])

    nc.compile()
    return nc


_CACHE = {}


def _get_program():
    if "nc" not in _CACHE:
        _CACHE["nc"] = build_program()
    return _CACHE["nc"]


def kernel(x, codebook, random_indices):
    x = np.ascontiguousarray(np.asarray(x, dtype=np.float32))
    codebook = np.ascontiguousarray(np.asarray(codebook, dtype=np.float32))
    ri = np.asarray(random_indices)

    nc = _get_program()

    # host-side marshalling: shard x, replicate codebook, build penalty vector
    xf = x.reshape(N, D)
    pen = np.full((1, NTOK), PENALTY, dtype=np.float32)
    pen[0, ri.astype(np.int64)] = 0.0
    cT = np.ascontiguousarray(codebook.T)

    in_maps = []
    for c in range(NCORES):
        xc = xf[c * NLOC:(c + 1) * NLOC]
        in_maps.append({
            "xT": np.ascontiguousarray(xc.T),
            "xn": np.ascontiguousarray(xc),
            "cT": cT,
            "cb": codebook,
            "pen": pen,
        })

    res = run_bass_kernel_spmd(nc, in_maps, core_ids=list(range(NCORES)))

    ctoks = np.concatenate([r["ctok"] for r in res.results], axis=0)
    closest_tokens = ctoks.reshape(B, S, D)
    cidx = np.concatenate([r["cidx"] for r in res.results], axis=0)
    closest_indices = cidx.reshape(B, S).astype(np.int32)
    total_sse = np.sum([r["lpart"][0, 0] for r in res.results], dtype=np.float64)
    rounding_loss = np.float32(total_sse / (B * S * D))
    return closest_tokens, rounding_loss, closest_indices
